# revision 18
# baseline (speedup 1.0000x reference)
"""Trainium2 Bass kernel for nn_DigitalPhaser (4-stage time-varying allpass
phaser with feedback; x: [64, 240000] f32).

Architecture (v2): pure batch parallelism -- 8 lanes per core, full T on
every core, ZERO collectives (the previous AllGather cost ~77us of a 152us
kernel).  The per-sample recurrence is linear time-varying in a 5-dim
minimal state s=(h1,h2,h3,h4,u), u[t]=x[t]+fb*h4[t-1]:

    s_t = M_t s_{t-1} + c_t x_t,   y_t = h4_t + x_t.

Chunked into L=120 samples: y_c = K_c x_c + U_c s_c ; s-chain via
d_c = G_c x_c.  K/U/G vary smoothly along the compile-time LFO schedule, so
they are fitted per group of 25 chunks (3000 samples, aligned to the LFO
triangle apexes at t = 24000k) as A0 + delta_c*A1; the linear term rides a
host-precomputed delta-scaled copy of x.  One group's [128,~120] stationary
then serves 200 moving columns (25 chunks x 8 lanes), vs 64 in the old
kernel, and the weight stream drops ~4x.

State recovery is core-local: the 25-chunk group propagator has norm
~1e-5, so each group's start state S_g equals the previous group's end
summary E_{g-1} exactly; within-group chunk states come from one exact
[128,125] matmul per group (contraction = 24 chunk-d's + S_g).

All coefficients are input-independent host constants.  Only x-derived
tensors (x, delta*x) and y cross HBM at runtime plus ~1MB of tiny state
reshuffles through DRAM scratch.
"""

import os
import numpy as np
import ml_dtypes

import concourse.bass as bass
import concourse.bacc as bacc
import concourse.mybir as mybir
from concourse.tile import TileContext
from concourse.bass_utils import run_bass_kernel_spmd

SAMPLE_RATE = 48000.0
F0 = 0.5
F_MIN = 1000.0
F_MAX = 4000.0
FB = 0.7

BFULL = 64
BL = 8                      # lanes per core
T = 240000
N_CORES = 8
L = 120                     # samples per chunk
C = T // L                  # 2000 chunks
GC = 25                     # chunks per interp group (3000 smp, apex-aligned)
NG = C // GC                # 80 groups
NCOL = C * BL               # 16000 moving columns, col = c*8 + lane
GCOL = GC * BL              # 200 columns per group

MODE = os.environ.get("BASS_PHASER_MODE", "f8")   # "f8" | "bf16"


# ---------------------------------------------------------------- host math
def _compute_p(n):
    t = np.arange(n, dtype=np.float32) / np.float32(SAMPLE_RATE)
    phase = np.float32(2.0 * np.pi * F0) * t
    frac = np.mod(phase / np.float32(2.0 * np.pi), np.float32(1.0))
    tri = np.where(frac < 0.5, 4.0 * frac - 1.0, 3.0 - 4.0 * frac).astype(np.float32)
    d_min = np.float32(F_MIN * 2.0 / SAMPLE_RATE)
    d_max = np.float32(F_MAX * 2.0 / SAMPLE_RATE)
    depth = np.float32((d_max - d_min) * 0.5)
    lfo = d_min + depth * (np.float32(1.0) + tri)
    tanl = np.tan(lfo.astype(np.float32))
    p = (np.float32(1.0) - tanl) / (np.float32(1.0) + tanl)
    return p.astype(np.float64)


def _build_Mc(p):
    """5-state one-step matrices; state order (h1,h2,h3,h4,u)."""
    n = p.shape[0]
    r_u = np.zeros((n, 5)); r_u[:, 3] = FB
    c_u = np.ones(n)
    r_h1 = p[:, None] * r_u; r_h1[:, 0] += p; r_h1[:, 4] -= 1.0
    c_h1 = p * c_u
    r_h2 = p[:, None] * r_h1; r_h2[:, 1] += p; r_h2[:, 0] -= 1.0
    c_h2 = p * c_h1
    r_h3 = p[:, None] * r_h2; r_h3[:, 2] += p; r_h3[:, 1] -= 1.0
    c_h3 = p * c_h2
    r_h4 = p[:, None] * r_h3; r_h4[:, 3] += p; r_h4[:, 2] -= 1.0
    c_h4 = p * c_h3
    M = np.stack([r_h1, r_h2, r_h3, r_h4, r_u], axis=1)
    c = np.stack([c_h1, c_h2, c_h3, c_h4, c_u], axis=1)
    return M, c


def _chunk_mats(p):
    """Per-chunk K [C,L,L] (with +I wet mix), U [C,L,5], G [C,5,L], P [C,5,5]."""
    M, c = _build_Mc(p)
    Mb = M.reshape(C, L, 5, 5)
    cb = c.reshape(C, L, 5)
    Phi = np.empty((C, L, 5, 5))
    Phi[:, 0] = Mb[:, 0]
    for r in range(1, L):
        Phi[:, r] = np.einsum('cij,cjk->cik', Mb[:, r], Phi[:, r - 1])
    K = np.zeros((C, L, L))
    G = np.zeros((C, 5, L))
    Tcur = cb.copy()
    for lag in range(L):
        qmax = L - lag
        idx = np.arange(qmax)
        K[:, idx + lag, idx] = Tcur[:, :qmax, 3]
        G[:, :, L - 1 - lag] = Tcur[:, L - 1 - lag, :]
        if lag < L - 1:
            nq = qmax - 1
            Tcur[:, :nq] = np.einsum('cqij,cqj->cqi', Mb[:, lag + 1:], Tcur[:, :nq])
    K[:, np.arange(L), np.arange(L)] += 1.0
    U = Phi[:, :, 3, :].copy()
    P = Phi[:, L - 1].copy()
    return K, U, G, P


def _precompute():
    p = _compute_p(T)
    K, U, G, P = _chunk_mats(p)
    delta = -1.0 + (2 * np.arange(GC) + 1) / GC          # per chunk in group
    V = np.vander(delta, 2, increasing=True)

    def gfit(A):
        A2 = A.reshape(NG, GC, -1).transpose(1, 0, 2).reshape(GC, -1)
        cth, *_ = np.linalg.lstsq(V, A2, rcond=None)
        return cth.reshape((2, NG) + A.shape[1:])

    Kc, Uc, Gc = gfit(K), gfit(U), gfit(G)

    # exact per-group state matrices
    I5 = np.eye(5)
    XiA = np.zeros((NG, 128, 128))      # rows: (m'=0..23,s)=0:120, S=120:125
    Wst = np.zeros((NG, 128, 8))        # rows: (m'=0..23,s)=0:120, d24=120:125
    for g in range(NG):
        Pg = P[g * GC:(g + 1) * GC]
        XiT = np.zeros((GC, 5, 5)); XiT[0] = I5
        for m in range(1, GC):
            XiT[m] = Pg[m - 1] @ XiT[m - 1]
        for m in range(GC):             # out cols (m,s) = m*5+s
            acc = I5
            for mp in range(m - 1, -1, -1):       # coef of d_{mp}
                if mp < GC - 1:
                    XiA[g, mp * 5:(mp + 1) * 5, m * 5:(m + 1) * 5] = acc.T
                acc = acc @ Pg[mp]
            XiA[g, 120:125, m * 5:(m + 1) * 5] = XiT[m].T
        acc = I5
        for mp in range(GC - 1, -1, -1):          # E_g = sum What[mp] d_mp
            if mp == GC - 1:
                Wst[g, 120:125, 0:5] = acc.T
            else:
                Wst[g, mp * 5:(mp + 1) * 5, 0:5] = acc.T
            acc = acc @ Pg[mp]

    # KU stationaries [128, L]: rows 0:120 K^T, 120:125 U^T, 125:128 zero
    KU0 = np.zeros((NG, 128, L)); KU1 = np.zeros((NG, 128, L))
    KU0[:, 0:L] = Kc[0].transpose(0, 2, 1); KU0[:, L:L + 5] = Uc[0].transpose(0, 2, 1)
    KU1[:, 0:L] = Kc[1].transpose(0, 2, 1); KU1[:, L:L + 5] = Uc[1].transpose(0, 2, 1)
    # G stationaries [128, 16]: cols 0:8 G0^T(pad), 8:16 G1^T(pad)
    Gst = np.zeros((NG, 128, 16))
    Gst[:, 0:L, 0:5] = Gc[0].transpose(0, 2, 1)
    Gst[:, 0:L, 8:13] = Gc[1].transpose(0, 2, 1)
    XiAt = XiA.copy()
    dcol = np.zeros(128)
    dcol[0:125] = np.repeat(delta, 5)
    XiAt *= dcol[None, None, :]
    return dict(KU0=KU0, KU1=KU1, Gst=Gst, XiA=XiA, XiAt=XiAt, Wst=Wst,
                delta=delta)


# ---------------------------------------------------------------- device
def _build_nc(mode):
    f32 = mybir.dt.float32
    bf16 = mybir.dt.bfloat16
    xdt = bf16 if mode == "bf16" else mybir.dt.float8e4

    nc = bacc.Bacc(num_devices=N_CORES)
    Par = lambda name, shape, dt: nc.declare_dram_parameter(
        name, list(shape), dt, isOutput=False)
    xT = Par("xT", (128, NCOL), bf16)
    xtT = Par("xtT", (128, NCOL), xdt)
    KU0 = Par("KU0", (128, NG * L), bf16)
    KU1 = Par("KU1", (128, NG * L), xdt)
    Gst = Par("Gst", (128, NG * 16), bf16)
    XiA = Par("XiA", (128, NG * 128), bf16)
    XiAt = Par("XiAt", (128, NG * 128), bf16)
    Wst = Par("Wst", (128, NG * 8), bf16)
    yT = nc.declare_dram_parameter("yT", [L, NCOL], bf16, isOutput=True)

    NSPL = 4                       # x loaded in 4 slices
    SCOL = NCOL // NSPL

    with TileContext(nc) as tc:
        with (
            tc.tile_pool(name="xin", bufs=1) as xp,
            tc.tile_pool(name="wts", bufs=1) as wp,
            tc.tile_pool(name="dsb", bufs=1) as dsp,
            tc.tile_pool(name="ysb", bufs=4) as yp,
            tc.tile_pool(name="ps_d", bufs=2, space="PSUM") as ps_d,
            tc.tile_pool(name="ps_e", bufs=1, space="PSUM") as ps_e,
            tc.tile_pool(name="ps_s", bufs=1, space="PSUM") as ps_s,
            tc.tile_pool(name="ps_y", bufs=3, space="PSUM") as ps_y,
            tc.tile_pool(name="dram", bufs=1, space="DRAM") as dp,
        ):
            # x/xt columns are GROUP-MINOR: col = (m*8+l)*80 + g, so that
            # every state reshuffle has 80-element contiguous runs.
            x_sb = xp.tile([128, NCOL], bf16, tag="x")
            xt_sb = xp.tile([128, NCOL], xdt, tag="xt")
            for i in range(NSPL):
                sl = slice(i * SCOL, (i + 1) * SCOL)
                nc.sync.dma_start(out=x_sb[:, sl], in_=xT[:, sl])
                nc.sync.dma_start(out=xt_sb[:, sl], in_=xtT[:, sl])
            x_v = x_sb[:, :].rearrange("p (v g) -> p g v", v=GCOL, g=NG)
            xt_v = xt_sb[:, :].rearrange("p (v g) -> p g v", v=GCOL, g=NG)

            def cload(param, cols, tag, dt):
                t = wp.tile([128, NG * cols], dt, tag=tag)
                nc.scalar.dma_start(out=t[:], in_=param[:, :])
                return t

            gst_t = cload(Gst, 16, "gst", bf16)
            ku0_t = cload(KU0, L, "ku0", bf16)
            ku1_t = cload(KU1, L, "ku1", xdt)
            xia_t = cload(XiA, 128, "xia", bf16)
            xit_t = cload(XiAt, 128, "xit", bf16)
            wst_t = cload(Wst, 8, "wst", bf16)

            # ---- D-pass -> d_sb [8, (v, g)] (same group-minor col order)
            d_sb = dsp.tile([8, NCOL], bf16, tag="dsb")
            d_v = d_sb[:, :].rearrange("p (v g) -> p g v", v=GCOL, g=NG)
            for g in range(NG):
                pd = ps_d.tile([8, GCOL], f32, tag="pd")
                nc.tensor.matmul(pd[:], gst_t[:, g * 16:g * 16 + 8],
                                 x_v[:, g], start=True, stop=False)
                nc.tensor.matmul(pd[:], gst_t[:, g * 16 + 8:g * 16 + 16],
                                 xt_v[:, g], start=False, stop=True)
                if g % 2 == 0:
                    nc.vector.tensor_copy(out=d_v[:, g], in_=pd[:])
                else:
                    nc.scalar.copy(out=d_v[:, g], in_=pd[:])

            # ---- D flip: d_sb [s, ((m,l), g)] -> dT [(m,s), (l, g)]
            d_dram = dp.tile([5, NCOL], bf16, tag="dda")
            nc.gpsimd.dma_start(out=d_dram[:, :], in_=d_sb[0:5, :])
            dT_sb = dsp.tile([128, NG * BL], bf16, tag="dT")
            nc.vector.memset(dT_sb[:, :], 0.0)
            dd_v = d_dram[:, :].rearrange("s (m l g) -> m s l g",
                                          m=GC, l=BL, g=NG)
            dt_m = dT_sb[0:125, :].rearrange("(m s) (l g) -> m s l g",
                                             m=GC, s=5, l=BL, g=NG)
            for m in range(GC):
                eng = (nc.sync, nc.scalar, nc.gpsimd)[m % 3]
                eng.dma_start(out=dt_m[m], in_=dd_v[m])
            dT_v = dT_sb[:, :].rearrange("p (l g) -> p g l", l=BL, g=NG)

            # ---- E_g = What_g . dT_g ; e_sb cols (l, g'=g+1); S = shift
            e_sb = dsp.tile([8, (NG + 1) * BL], bf16, tag="esb")
            e_v = e_sb[:, :].rearrange("p (l q) -> p q l", l=BL, q=NG + 1)
            nc.vector.memset(e_v[:, 0], 0.0)
            for h in range(2):
                pe = ps_e.tile([8, 40 * BL], f32, tag="pe")
                for q in range(40):
                    g = h * 40 + q
                    nc.tensor.matmul(pe[:, q * BL:(q + 1) * BL],
                                     wst_t[:, g * 8:(g + 1) * 8],
                                     dT_v[:, g], start=True, stop=True)
                nc.vector.tensor_copy(
                    out=e_v[:, 1 + h * 40: 1 + (h + 1) * 40],
                    in_=pe[:].rearrange("p (q l) -> p q l", q=40, l=BL))
            nc.gpsimd.dma_start(
                out=dT_sb[120:125, :].rearrange("s (l g) -> s l g",
                                                l=BL, g=NG),
                in_=e_sb[0:5, :].rearrange("s (l q) -> s l q",
                                           l=BL, q=NG + 1)[:, :, 0:NG])

            # ---- within-group recon; psum cols (g-half, l); evict to (l,g)
            s_sb = dsp.tile([128, NG * BL], bf16, tag="ssb")
            st_sb = dsp.tile([128, NG * BL], bf16, tag="stsb")
            s_v = s_sb[:, :].rearrange("p (l g) -> p g l", l=BL, g=NG)
            st_v = st_sb[:, :].rearrange("p (l g) -> p g l", l=BL, g=NG)
            for h in range(2):
                ps = ps_s.tile([128, 40 * BL], f32, tag="ps")
                pst = ps_e.tile([128, 40 * BL], f32, tag="pst")
                for q in range(40):
                    g = h * 40 + q
                    nc.tensor.matmul(ps[:, q * BL:(q + 1) * BL],
                                     xia_t[:, g * 128:(g + 1) * 128],
                                     dT_v[:, g], start=True, stop=True)
                    nc.tensor.matmul(pst[:, q * BL:(q + 1) * BL],
                                     xit_t[:, g * 128:(g + 1) * 128],
                                     dT_v[:, g], start=True, stop=True)
                pr = ps[:].rearrange("p (q l) -> p q l", q=40, l=BL)
                ptr = pst[:].rearrange("p (q l) -> p q l", q=40, l=BL)
                nc.vector.tensor_copy(out=s_v[:, h * 40:(h + 1) * 40], in_=pr)
                nc.scalar.copy(out=st_v[:, h * 40:(h + 1) * 40], in_=ptr)

            # ---- inject states into x/xt rows 120:125 (per-m, 80-el runs)
            si_m = x_sb[120:125, :].rearrange("s (m l g) -> m s l g",
                                              m=GC, l=BL, g=NG)
            ti_m = xt_sb[120:125, :].rearrange("s (m l g) -> m s l g",
                                               m=GC, l=BL, g=NG)
            ss_m = s_sb[0:125, :].rearrange("(m s) (l g) -> m s l g",
                                            m=GC, s=5, l=BL, g=NG)
            ts_m = st_sb[0:125, :].rearrange("(m s) (l g) -> m s l g",
                                             m=GC, s=5, l=BL, g=NG)
            for m in range(GC):
                nc.gpsimd.dma_start(out=si_m[m], in_=ss_m[m])
                nc.gpsimd.dma_start(out=ti_m[m], in_=ts_m[m])

            # ---- Y-pass
            for b in range(NG // 2):
                py = ps_y.tile([L, 2 * GCOL], f32, tag="py")
                for j in range(2):
                    g = b * 2 + j
                    ps_sl = slice(j * GCOL, (j + 1) * GCOL)
                    nc.tensor.matmul(py[:, ps_sl], ku0_t[:, g * L:(g + 1) * L],
                                     x_v[:, g], start=True, stop=False)
                    nc.tensor.matmul(py[:, ps_sl], ku1_t[:, g * L:(g + 1) * L],
                                     xt_v[:, g], start=False, stop=True)
                yt = yp.tile([L, 2 * GCOL], bf16, tag="yt")
                if b % 2 == 0:
                    nc.vector.tensor_copy(out=yt[:], in_=py[:])
                else:
                    nc.scalar.copy(out=yt[:], in_=py[:])
                eng = nc.sync if b % 2 == 0 else nc.scalar
                eng.dma_start(out=yT[:, b * 2 * GCOL:(b + 1) * 2 * GCOL],
                              in_=yt[:])

    nc.compile()
    return nc


# ---------------------------------------------------------------- driver
_CACHE = {}


def _get_built(mode):
    if mode not in _CACHE:
        coef = _precompute()
        bfdt = ml_dtypes.bfloat16
        xdt = bfdt if mode == "bf16" else ml_dtypes.float8_e4m3fn
        def pk(a, dt):
            g, p, c = a.shape
            return np.ascontiguousarray(
                a.transpose(1, 0, 2).reshape(p, g * c).astype(dt))
        base = dict(
            KU0=pk(coef['KU0'], bfdt),
            KU1=pk(coef['KU1'], xdt),
            Gst=pk(coef['Gst'], bfdt),
            XiA=pk(coef['XiA'], bfdt),
            XiAt=pk(coef['XiAt'], bfdt),
            Wst=pk(coef['Wst'], bfdt),
        )
        nc = _build_nc(mode)
        _CACHE[mode] = (nc, base, coef['delta'], xdt)
    return _CACHE[mode]


def _run(x, mode, trace=False):
    nc, base, delta, xdt = _get_built(mode)
    x = np.asarray(x, dtype=np.float32)
    dfull = np.tile(delta, NG).astype(np.float32)        # [C]
    in_maps = []
    for k in range(N_CORES):
        xb = x[k * BL:(k + 1) * BL]                      # [8, 240000]
        # col = ((m*8 + l) * 80) + g  (group-minor)
        xc = xb.reshape(BL, NG, GC, L).transpose(3, 2, 0, 1)  # [tau, m, l, g]
        xrow = np.zeros((128, NCOL), np.float32)
        xrow[0:L] = xc.reshape(L, NCOL)
        dsc = dfull.reshape(NG, GC).transpose(1, 0)      # [m, g]
        xtrow = np.zeros((128, NCOL), np.float32)
        xtrow[0:L] = (xc * dsc[None, :, None, :]).reshape(L, NCOL)
        m = dict(base)
        m["xT"] = np.ascontiguousarray(xrow.astype(ml_dtypes.bfloat16))
        m["xtT"] = np.ascontiguousarray(xtrow.astype(xdt))
        in_maps.append(m)
    res = run_bass_kernel_spmd(nc, in_maps, list(range(N_CORES)), trace=trace)
    y = np.empty((BFULL, T), np.float32)
    for k in range(N_CORES):
        yT = np.asarray(res.results[k]["yT"]).astype(np.float32)
        # yT cols: (g-block, v=(m,l)): [tau, g, m, l]
        y[k * BL:(k + 1) * BL] = (yT.reshape(L, NG, GC, BL)
                                  .transpose(3, 1, 2, 0).reshape(BL, T))
    return y, res


def kernel(x):
    y, _ = _run(x, MODE, trace=False)
    return y


def run_traced(x, mode=MODE):
    return _run(x, mode, trace=True)


# revision 19
# speedup vs baseline: 1.8633x; 1.8633x over previous
"""Trainium2 Bass kernel for nn_DigitalPhaser (4-stage time-varying allpass
phaser with feedback; x: [64, 240000] f32).

The per-sample recurrence is linear time-varying in an 8-dim state
(s_t = M_t s_{t-1} + c_t x_t, y_t = s_t[6] + x_t) with input-independent
M_t/c_t, so the scan factors into host-precomputed coefficient matrices
and on-device matmuls:

  - time sharded across 8 cores (zero-pad 240000 -> 245760, 30720/core);
    every core keeps all 64 lanes so matmuls get a 64-wide moving operand;
  - chunks of L=120 samples; per chunk the contraction is augmented to
    128 = 120 x-samples + 8 state entries, so one fused matmul computes
    Y = tril(K) @ X + U @ s_start with a [128,128] stationary (KU);
  - chunk start-states recovered hierarchically (16 chunks/superchunk,
    16 superchunks/core) from d_j = G_j X_j via host-precomposed 8x8
    propagator products;
  - the only cross-core dependency (each core's start state) is an
    AllGather of one 8x64 tile, then a per-core precomposed mix.

Coefficients depend only on the compile-time LFO schedule: computed here
in float64, shipped as per-core kernel inputs.
"""

import os
import numpy as np
import ml_dtypes

import concourse.bass as bass
import concourse.bacc as bacc
import concourse.mybir as mybir
from concourse.tile import TileContext
from concourse.bass_utils import run_bass_kernel_spmd

SAMPLE_RATE = 48000.0
F0 = 0.5
F_MIN = 1000.0
F_MAX = 4000.0
FB = 0.7

B = 64
T = 240000
T_PAD = 245760
N_CORES = 8
T_C = T_PAD // N_CORES     # 30720
L = 120                    # samples per chunk (contraction 120+8 states)
C_C = T_C // L             # 256 chunks / core
Q = 16                     # chunks / superchunk
N_SQ = C_C // Q            # 16
N_CH = T_PAD // L          # 2048
GB = 8                     # chunks per PSUM group (one 2KB bank)
NG = C_C // GB             # 32 psum groups
GD = 16                    # chunks per DMA group / SBUF tile
ND = C_C // GD             # 16 DMA groups

MODE = os.environ.get("BASS_PHASER_MODE", "bf16")  # "f32" | "bf16"


# ---------------------------------------------------------------- host math
def _compute_p(n):
    t = np.arange(n, dtype=np.float32) / np.float32(SAMPLE_RATE)
    phase = np.float32(2.0 * np.pi * F0) * t
    frac = np.mod(phase / np.float32(2.0 * np.pi), np.float32(1.0))
    tri = np.where(frac < 0.5, 4.0 * frac - 1.0, 3.0 - 4.0 * frac).astype(np.float32)
    d_min = np.float32(F_MIN * 2.0 / SAMPLE_RATE)
    d_max = np.float32(F_MAX * 2.0 / SAMPLE_RATE)
    depth = np.float32((d_max - d_min) * 0.5)
    lfo = d_min + depth * (np.float32(1.0) + tri)
    tanl = np.tan(lfo.astype(np.float32))
    p = (np.float32(1.0) - tanl) / (np.float32(1.0) + tanl)
    return p.astype(np.float64)


def _build_Mc(p):
    n = p.shape[0]
    M = np.zeros((n, 8, 8))
    c = np.zeros((n, 8))
    r0 = np.zeros((n, 8)); r0[:, 0] = p; r0[:, 1] = -1; r0[:, 6] = p * FB
    c0 = p
    r1 = np.zeros((n, 8)); r1[:, 6] = FB
    c1 = np.ones(n)
    r2 = p[:, None] * r0; r2[:, 2] += p; r2[:, 3] -= 1
    c2 = p * c0
    r4 = p[:, None] * r2; r4[:, 4] += p; r4[:, 5] -= 1
    c4 = p * c2
    r6 = p[:, None] * r4; r6[:, 6] += p; r6[:, 7] -= 1
    c6 = p * c4
    for i, (r, cc) in enumerate([(r0, c0), (r1, c1), (r2, c2), (r0, c0),
                                 (r4, c4), (r2, c2), (r6, c6), (r4, c4)]):
        M[:, i, :] = r
        c[:, i] = cc
    return M, c


def _precompute():
    p64 = _compute_p(T_PAD)
    M, c = _build_Mc(p64)
    Mb = M.reshape(N_CH, L, 8, 8)
    cb = c.reshape(N_CH, L, 8)

    Phi = np.empty((N_CH, L, 8, 8))
    Phi[:, 0] = Mb[:, 0]
    for r in range(1, L):
        Phi[:, r] = Mb[:, r] @ Phi[:, r - 1]

    K = np.zeros((N_CH, L, L))
    G = np.zeros((N_CH, 8, L))
    Tcur = cb.copy()
    for lag in range(L):
        qmax = L - lag
        idx = np.arange(qmax)
        K[:, idx + lag, idx] = Tcur[:, :qmax, 6]
        G[:, :, L - 1 - lag] = Tcur[:, L - 1 - lag, :]
        if lag < L - 1:
            nq = qmax - 1
            Tcur[:, :nq] = np.einsum('nqij,nqj->nqi', Mb[:, lag + 1:], Tcur[:, :nq])
    K[:, np.arange(L), np.arange(L)] += 1.0      # wet-mix identity on the diag

    U = Phi[:, :, 6, :].copy()                   # [N_CH, L, 8]
    P = Phi[:, L - 1].copy()

    Pc = P.reshape(N_CORES, C_C, 8, 8)
    What = np.zeros((N_CORES, N_SQ, Q, 8, 8))
    Xi_T = np.zeros((N_CORES, N_SQ, Q, 8, 8))
    Xi_D = np.zeros((N_CORES, N_SQ, Q, Q, 8, 8))
    R = np.zeros((N_CORES, N_SQ, 8, 8))
    I8 = np.eye(8)
    for k in range(N_CORES):
        for q in range(N_SQ):
            Pq = Pc[k, q * Q:(q + 1) * Q]
            V = np.zeros((Q, 8, 8)); V[0] = I8
            for m in range(1, Q):
                V[m] = Pq[m - 1] @ V[m - 1]
            Xi_T[k, q] = V
            for m in range(Q):
                acc = I8
                for mp in range(m - 1, -1, -1):
                    Xi_D[k, q, m, mp] = acc
                    acc = acc @ Pq[mp]
            acc = I8
            for m in range(Q - 1, -1, -1):
                What[k, q, m] = acc
                acc = acc @ Pq[m]
            R[k, q] = acc

    Lam = np.zeros((N_CORES, N_SQ, 1 + N_SQ, 8, 8))
    Gam = np.zeros((N_CORES, 1 + N_SQ, 8, 8))
    Z = np.zeros((N_CORES, 8, 8))
    for k in range(N_CORES):
        RV = np.zeros((N_SQ + 1, 8, 8)); RV[0] = I8
        for q in range(1, N_SQ + 1):
            RV[q] = R[k, q - 1] @ RV[q - 1]
        Z[k] = RV[N_SQ]
        for q in range(N_SQ):
            Lam[k, q, 0] = RV[q]
            acc = I8
            for qp in range(q - 1, -1, -1):
                Lam[k, q, 1 + qp] = acc
                acc = acc @ R[k, qp]
        acc = I8
        for qp in range(N_SQ - 1, -1, -1):
            Gam[k, 1 + qp] = acc
            acc = acc @ R[k, qp]

    Theta = np.zeros((N_CORES, N_CORES, 8, 8))
    for k in range(N_CORES):
        acc = I8
        for j in range(k - 1, -1, -1):
            Theta[k, j] = acc
            acc = acc @ Z[j]

    return dict(K=K, U=U, G=G, What=What, Xi_T=Xi_T, Xi_D=Xi_D,
                Lam=Lam, Gam=Gam, Theta=Theta)


def _pack_core(coef, k, np_dt):
    sl = slice(k * C_C, (k + 1) * C_C)
    KU = np.zeros((C_C, 128, 128))
    KU[:, 0:L, 0:L] = coef['K'][sl].transpose(0, 2, 1)       # K^T: [tau, t]
    KU[:, L:128, 0:L] = coef['U'][sl].transpose(0, 2, 1)     # U^T: [k, t]
    Kt16 = (KU.reshape(ND, GD, 128, 128).transpose(0, 2, 1, 3)
            .reshape(ND, 128, GD * 128))

    Gt = (coef['G'][sl].reshape(N_SQ, Q, 8, L)
          .transpose(3, 0, 1, 2).reshape(L, N_SQ * Q * 8))
    Wh = coef['What'][k].transpose(1, 3, 0, 2).reshape(Q * 8, N_SQ * 8)
    Gm = coef['Gam'][k, 1:].transpose(2, 0, 1).reshape(8, N_SQ * 8)
    Th = coef['Theta'][k].transpose(0, 2, 1).reshape(N_CORES * 8, 8)
    LmS = coef['Lam'][k, :, 0].transpose(2, 0, 1).reshape(8, N_SQ * 8)
    LmE = (coef['Lam'][k, :, 1:].transpose(3, 1, 0, 2)
           .reshape(8, N_SQ * 128))
    XiT = coef['Xi_T'][k].transpose(3, 0, 1, 2).reshape(8, N_SQ * Q * 8)
    XiD = coef['Xi_D'][k].transpose(2, 4, 0, 1, 3).reshape(Q * 8, N_SQ * Q * 8)
    ident = np.eye(B)
    out = dict(Kt16=Kt16, Gt=Gt, Wh=Wh, Gm=Gm, Th=Th, LmS=LmS, LmE=LmE,
               XiT=XiT, XiD=XiD, ident=ident)
    return {n: np.ascontiguousarray(a.astype(np_dt)) for n, a in out.items()}


# ---------------------------------------------------------------- device
def _build_nc(mode):
    f32 = mybir.dt.float32
    dt = f32 if mode == "f32" else mybir.dt.bfloat16
    from concourse.tile_rust import add_dep_helper

    nc = bacc.Bacc(num_devices=N_CORES)
    P_ = lambda name, shape: nc.declare_dram_parameter(name, list(shape), dt,
                                                       isOutput=False)
    xT16 = P_("xT16", (ND, L, GD * B))
    Kt16 = P_("Kt16", (ND, 128, GD * 128))
    Gt = P_("Gt", (L, N_SQ * Q * 8))
    Wh = P_("Wh", (Q * 8, N_SQ * 8))
    Gm = P_("Gm", (8, N_SQ * 8))
    Th = P_("Th", (N_CORES * 8, 8))
    LmS = P_("LmS", (8, N_SQ * 8))
    LmE = P_("LmE", (8, N_SQ * 128))
    XiT = P_("XiT", (8, N_SQ * Q * 8))
    XiD = P_("XiD", (Q * 8, N_SQ * Q * 8))
    ident = P_("ident", (B, B))
    yT16 = nc.declare_dram_parameter("yT16", [ND, L, GD * B], dt, isOutput=True)

    with TileContext(nc) as tc:
        with (
            tc.tile_pool(name="const", bufs=1) as cp,
            tc.tile_pool(name="xres", bufs=1) as xpool,
            tc.tile_pool(name="kst", bufs=1) as kp,
            tc.tile_pool(name="yst", bufs=4) as yp,
            tc.tile_pool(name="svp", bufs=2) as svp,
            tc.tile_pool(name="ps_y", bufs=3, space="PSUM") as ps_y,
            tc.tile_pool(name="ps_8", bufs=1, space="PSUM") as ps_8,
            tc.tile_pool(name="ps_v", bufs=2, space="PSUM") as ps_v,
            tc.tile_pool(name="ps_a", bufs=1, space="PSUM") as ps_a,
            tc.tile_pool(name="ps_t", bufs=1, space="PSUM") as ps_t,
            tc.tile_pool(name="dram", bufs=1, space="DRAM") as dp,
        ):
            def cload(param, shape, tag):
                t = cp.tile(list(shape), dt, tag=tag)
                nc.sync.dma_start(out=t[:], in_=param[:, :])
                return t

            # x loads first, split across two issue queues
            xg = []
            x_dmas = []
            for g in range(ND):
                t = xpool.tile([128, GD * B], dt, tag=f"x{g}")
                eng = nc.sync if g % 2 == 0 else nc.gpsimd
                x_dmas.append(eng.dma_start(out=t[0:L, :], in_=xT16[g, :, :]))
                xg.append(t)

            gt_t = cload(Gt, (L, N_SQ * Q * 8), "gt")
            wh_t = cload(Wh, (Q * 8, N_SQ * 8), "wh")
            gm_t = cload(Gm, (8, N_SQ * 8), "gm")
            th_t = cload(Th, (N_CORES * 8, 8), "th")
            lms_t = cload(LmS, (8, N_SQ * 8), "lms")
            lme_t = cload(LmE, (8, N_SQ * 128), "lme")
            xit_t = cload(XiT, (8, N_SQ * Q * 8), "xit")
            xid_t = cload(XiD, (Q * 8, N_SQ * Q * 8), "xid")
            id_t = cload(ident, (B, B), "id")

            s_t = cp.tile([8, B], dt, tag="s")              # core start state
            tvT_t = cp.tile([8, N_SQ * B], dt, tag="tvT")   # T_q along free dim
            fall_t = cp.tile([N_CORES * 8, B], dt, tag="fall")
            f_t = cp.tile([8, B], dt, tag="f")

            # KU weight loads on the scalar queue, SBUF-resident; gated
            # behind the x stream so x gets full HBM bandwidth first
            kg = []
            for g in range(ND):
                kt = kp.tile([128, GD * 128], dt, tag=f"k{g}")
                kd = nc.scalar.dma_start(out=kt[:], in_=Kt16[g, :, :])
                add_dep_helper(kd.ins, x_dmas[-1].ins, sync=True,
                               reason="throttle KU stream behind x loads")
                kg.append(kt)

            # ---- phase A: d_j = G_j X_j (transposed out), 16 chunks/psum
            dq_tiles = []
            for q in range(N_SQ):
                pd = ps_a.tile([B, Q * 8], f32, tag="pa")
                for m in range(Q):
                    j = q * Q + m
                    g, cc = j // GD, j % GD
                    nc.tensor.matmul(
                        pd[:, m * 8:(m + 1) * 8],
                        xg[g][0:L, cc * B:(cc + 1) * B],
                        gt_t[:, q * 128 + m * 8: q * 128 + (m + 1) * 8],
                        start=True, stop=True)
                dts = svp.tile([B, Q * 8], dt, tag="dts")
                nc.vector.tensor_copy(out=dts[:], in_=pd[:])
                ptr = ps_t.tile([Q * 8, B], dt, tag="ptr")
                nc.tensor.transpose(ptr[:], dts[:], id_t[:])
                dqt = cp.tile([Q * 8, B], dt, tag=f"d{q}")
                nc.vector.tensor_copy(out=dqt[:], in_=ptr[:])
                dq_tiles.append(dqt)

            # ---- E_q = What_q @ D_q  (base-0 tiles)
            e_parts = []
            for q in range(N_SQ):
                pe = ps_8.tile([8, B], f32, tag="p8")
                nc.tensor.matmul(pe[:], wh_t[:, q * 8:(q + 1) * 8],
                                 dq_tiles[q][:], start=True, stop=True)
                ep = cp.tile([8, B], dt, tag=f"e{q}")
                nc.vector.tensor_copy(out=ep[:], in_=pe[:])
                e_parts.append(ep)

            # ---- F = sum_q Gam_q @ E_q ; AllGather ; S = Theta_k @ F_all
            pf = ps_8.tile([8, B], f32, tag="p8")
            for q in range(N_SQ):
                nc.tensor.matmul(pf[:], gm_t[:, q * 8:(q + 1) * 8],
                                 e_parts[q][:],
                                 start=(q == 0), stop=(q == N_SQ - 1))
            nc.vector.tensor_copy(out=f_t[:], in_=pf[:])
            f_dram = dp.tile([8, B], dt, tag="fd")
            fall_dram = dp.tile([N_CORES * 8, B], dt, tag="fad")
            nc.gpsimd.dma_start(out=f_dram[:], in_=f_t[:])
            nc.gpsimd.collective_compute(
                "AllGather", mybir.AluOpType.bypass,
                replica_groups=[list(range(N_CORES))],
                ins=[f_dram[:]], outs=[fall_dram[:]])
            nc.gpsimd.dma_start(out=fall_t[:], in_=fall_dram[:])
            psk = ps_8.tile([8, B], f32, tag="p8")
            nc.tensor.matmul(psk[:], th_t[:], fall_t[:], start=True, stop=True)
            nc.vector.tensor_copy(out=s_t[:], in_=psk[:])

            # ---- Tvec = LamS @ S + sum LamE_q @ E_q ; reshuffle via HBM
            ptv = ps_v.tile([N_SQ * 8, B], f32, tag="pv")
            nc.tensor.matmul(ptv[:], lms_t[:], s_t[:], start=True, stop=False)
            for qp in range(N_SQ):
                nc.tensor.matmul(ptv[:], lme_t[:, qp * 128:(qp + 1) * 128],
                                 e_parts[qp][:],
                                 start=False, stop=(qp == N_SQ - 1))
            tvs = svp.tile([N_SQ * 8, B], dt, tag="tvs")
            nc.vector.tensor_copy(out=tvs[:], in_=ptv[:])
            tv_dram = dp.tile([N_SQ * 8, B], dt, tag="tvd")
            nc.gpsimd.dma_start(out=tv_dram[:], in_=tvs[:])
            nc.gpsimd.dma_start(
                out=tvT_t[:].rearrange("i (q l) -> i q l", q=N_SQ),
                in_=tv_dram[:].rearrange("(q i) l -> i q l", q=N_SQ, i=8))

            # ---- Svec_q = XiT @ T_q + XiD @ D_q ; ship to HBM scratch
            # (the XiD part only needs local D -- precompute before S arrives)
            svloc = []
            for q in range(N_SQ):
                pv = ps_v.tile([Q * 8, B], f32, tag="pv")
                nc.tensor.matmul(pv[:], xid_t[:, q * 128:(q + 1) * 128],
                                 dq_tiles[q][:], start=True, stop=True)
                sl_t = cp.tile([Q * 8, B], f32, tag=f"svl{q}")
                nc.vector.tensor_copy(out=sl_t[:], in_=pv[:])
                svloc.append(sl_t)
            sv_drams = []
            for q in range(N_SQ):
                pv = ps_v.tile([Q * 8, B], f32, tag="pv")
                nc.tensor.matmul(pv[:], xit_t[:, q * 128:(q + 1) * 128],
                                 tvT_t[:, q * B:(q + 1) * B],
                                 start=True, stop=True)
                svs = svp.tile([Q * 8, B], dt, tag="svs")
                nc.vector.tensor_tensor(out=svs[:], in0=pv[:],
                                        in1=svloc[q][:],
                                        op=mybir.AluOpType.add)
                svd = dp.tile([Q * 8, B], dt, tag=f"svd{q}")
                nc.gpsimd.dma_start(out=svd[:], in_=svs[:])
                sv_drams.append(svd)

            # ---- inject states into Xaug rows 120:128 (sync queue, idle now)
            for g in range(ND):
                nc.sync.dma_start(
                    out=xg[g][L:128, :].rearrange("k (c l) -> k c l", c=GD),
                    in_=sv_drams[g][:].rearrange("(c k) l -> k c l", c=GD, k=8))

            # ---- phase C: Y_j = KU_j @ [X_j; s_j]
            # 8 chunks accumulate into one PSUM bank -> single wide copy;
            # copies + stores alternate DVE / ACT by DMA group
            for g in range(ND):
                kt = kg[g]
                ceng = nc.scalar if g % 3 == 2 else nc.vector
                yt = yp.tile([L, GD * B], dt, tag="y")
                for h in range(GD // GB):                 # 2 psum groups
                    py = ps_y.tile([128, GB * B], f32, tag="py")
                    for c8 in range(GB):
                        cc = h * GB + c8
                        nc.tensor.matmul(
                            py[:, c8 * B:(c8 + 1) * B],
                            kt[:, cc * 128:(cc + 1) * 128],
                            xg[g][:, cc * B:(cc + 1) * B],
                            start=True, stop=True)
                    dst = yt[:, h * GB * B:(h + 1) * GB * B]
                    if ceng is nc.scalar:
                        nc.scalar.copy(out=dst, in_=py[0:L, :])
                    else:
                        nc.vector.tensor_copy(out=dst, in_=py[0:L, :])
                seng = nc.scalar if ceng is nc.scalar else nc.sync
                seng.dma_start(out=yT16[g, :, :], in_=yt[:])

    nc.compile()
    return nc


# ---------------------------------------------------------------- driver
_CACHE = {}


def _get_built(mode):
    if mode not in _CACHE:
        coef = _precompute()
        np_dt = np.float32 if mode == "f32" else ml_dtypes.bfloat16
        packed = [_pack_core(coef, k, np_dt) for k in range(N_CORES)]
        nc = _build_nc(mode)
        _CACHE[mode] = (nc, packed, np_dt)
    return _CACHE[mode]


def _run(x, mode, trace=False):
    nc, packed, np_dt = _get_built(mode)
    xp = np.zeros((B, T_PAD), np.float32)
    xp[:, :T] = np.asarray(x, dtype=np.float32)
    in_maps = []
    for k in range(N_CORES):
        xc = xp[:, k * T_C:(k + 1) * T_C].T                 # [30720, 64]
        xT16 = (xc.reshape(ND, GD, L, B).transpose(0, 2, 1, 3)
                .reshape(ND, L, GD * B))
        m = dict(packed[k])
        m["xT16"] = np.ascontiguousarray(xT16.astype(np_dt))
        in_maps.append(m)
    res = run_bass_kernel_spmd(nc, in_maps, list(range(N_CORES)), trace=trace)
    y = np.empty((B, T_PAD), np.float32)
    for k in range(N_CORES):
        yT16 = np.asarray(res.results[k]["yT16"]).astype(np.float32)
        yc = yT16.reshape(ND, L, GD, B).transpose(0, 2, 1, 3).reshape(T_C, B)
        y[:, k * T_C:(k + 1) * T_C] = yc.T
    return y[:, :T].astype(np.float32), res


def kernel(x):
    y, _ = _run(x, MODE, trace=False)
    return y


def run_traced(x, mode=MODE):
    return _run(x, mode, trace=True)

